# revision 1
# baseline (speedup 1.0000x reference)
"""Trainium2 Bass kernel for nn_Attention_29326036697518.

Dense spatial self-attention block (GroupNorm -> QKV 1x1conv -> HW x HW
attention -> out-proj -> residual) over x[32, 512, 32, 32].

Sharding: pure data-parallel over the batch dim — 4 batch elements per
NeuronCore, weights replicated, no collectives.

Per-core layout (per batch element, N = H*W = 1024, C = 512):
  x, out              : [C, N] as 4 partition-tiles [128, N]   (fp32)
  h, Q, K, h2         : [C, N] as 4 partition-tiles [128, N]   (bf16)
  V^T                 : [N, C] as 8 partition-tiles [128, C]   (bf16)
  P^T = exp(S^T)      : [N, N] as 8 partition-tiles [128, N]   (bf16)

Transpose-free attention: scores are computed directly transposed
(S^T = K_m^T Q per key-chunk), exp writes P^T in place, softmax row-sums
come from ones-vector matmuls over the partition dim (replicated across
partitions), and the 1/rowsum normalization is folded into the h2
PSUM->SBUF copy. GroupNorm group statistics are reduced across
channel-partitions with tiny indicator matmuls; stats/apply are split
across the vector and scalar engines. All heavy matmuls are bf16
(exact fp32 accumulation in PSUM); the residual path stays fp32.
"""

import sys

if "/opt/trn_rl_repo" not in sys.path:
    sys.path.insert(0, "/opt/trn_rl_repo")

import numpy as np

import concourse.bass as bass
import concourse.tile as tile
from concourse import bacc, mybir
from concourse.bass_utils import run_bass_kernel_spmd

F32 = mybir.dt.float32
F32R = mybir.dt.float32r
BF16 = mybir.dt.bfloat16

N_CORES = 8
B, C, H, W = 32, 512, 32, 32
HW = H * W                    # 1024
NB = B // N_CORES             # 4 batch elements per core
CT = C // 128                 # 4 channel partition-tiles
QC = HW // 128                # 8 spatial partition-tiles
G = 32                        # groupnorm groups
GS = C // G                   # 16 channels per group
EPS = 1e-5
SCALE = float(C) ** -0.5


def _build_body(nc, tc, ext):
    x_e, out_e = ext["x"], ext["out"]

    pools = {}
    def pool(name, bufs, space="SBUF"):
        pools[name] = tc.alloc_tile_pool(name=name, bufs=bufs, space=space)
        return pools[name]

    constp = pool("const", 1)
    wtsp = pool("wts", 1)
    xp = pool("xp", 2)
    hp = pool("hp", 2)
    qp = pool("qp", 1)
    kp = pool("kp", 1)
    vp = pool("vp", 1)
    ptp = pool("ptp", 1)
    h2p = pool("h2p", 1)
    outp = pool("outp", 2)
    rbp = pool("rbp", 2)
    gnp = pool("gnp", 2)
    ps_mm = pool("ps_mm", 3, space="PSUM")
    ps_sm = pool("ps_sm", 2, space="PSUM")

    def load_x(b):
        x_t = xp.tile([128, CT, HW], F32, tag="x", name="x_t")
        for t in range(CT):
            nc.sync.dma_start(out=x_t[:, t, :],
                              in_=x_e[b, 128 * t:128 * (t + 1), :])
        return x_t

    def gn_stats(x_t):
        """Groupnorm coefficients a,d; stats on DVE (tiles 0,1) + ACT (2,3),
        then group-reduce/broadcast matmuls, then h = a*x + d."""
        stat2 = gnp.tile([128, 2, 2], F32, tag="stat2", name="stat2")
        stat2b = gnp.tile([128, 2, 2], F32, tag="stat2b", name="stat2b")
        for t in range(2):
            st = gnp.tile([128, 2, 6], F32, tag="bnst", name="st")
            xin = x_t[:, t, :].rearrange("p (s f) -> p s f", f=512)
            for s in range(2):
                nc.vector.bn_stats(out=st[:, s, :], in_=xin[:, s, :])
            mv = gnp.tile([128, 2], F32, tag="bnmv", name="mv")
            nc.vector.bn_aggr(out=mv[:, :], in_=st[:, :, :])
            nc.vector.tensor_copy(stat2[:, t, 0:1], mv[:, 0:1])
            nc.vector.tensor_mul(stat2[:, t, 1:2], mv[:, 0:1], mv[:, 0:1])
            nc.vector.tensor_add(stat2[:, t, 1:2], stat2[:, t, 1:2], mv[:, 1:2])
        for t in range(2, CT):
            junk = gnp.tile([128, HW], BF16, tag="junk", name="junk")
            nc.scalar.activation(
                out=junk[:, :], in_=x_t[:, t, :],
                func=mybir.ActivationFunctionType.Identity,
                accum_out=stat2b[:, t - 2, 0:1])
            nc.scalar.activation(
                out=junk[:, :], in_=x_t[:, t, :],
                func=mybir.ActivationFunctionType.Square,
                accum_out=stat2b[:, t - 2, 1:2])

        psg = ps_sm.tile([G, 2], F32, tag="sm", name="psg")
        for t in range(CT):
            s2 = stat2[:, t, :] if t < 2 else stat2b[:, t - 2, :]
            nc.tensor.matmul(
                psg[:, :], indT_s[:, t, :], s2,
                start=(t == 0), stop=(t == CT - 1),
            )
        gsb = gnp.tile([G, 2], F32, tag="gsb", name="gsb")
        nc.vector.tensor_copy(gsb[:, :], psg[:, :])
        grp = gnp.tile([G, 2], F32, tag="grp", name="grp")
        nc.vector.tensor_copy(grp[:, 0:1], gsb[:, 0:1])
        tmp = gnp.tile([G, 1], F32, tag="gtmp", name="tmp")
        nc.vector.tensor_mul(tmp[:, :], gsb[:, 0:1], gsb[:, 0:1])
        nc.vector.tensor_sub(tmp[:, :], gsb[:, 1:2], tmp[:, :])
        nc.scalar.activation(tmp[:, :], tmp[:, :],
                             mybir.ActivationFunctionType.Sqrt,
                             bias=eps_t[:, :])
        nc.vector.reciprocal(grp[:, 1:2], tmp[:, :])

        ad = gnp.tile([128, CT, 2], F32, tag="ad", name="ad")
        for t in range(CT):
            psc = ps_sm.tile([128, 2], F32, tag="sm", name="psc")
            nc.tensor.matmul(psc[:, :], ind2_s[:, t, :], grp[:, :],
                             start=True, stop=True)
            nc.vector.tensor_mul(ad[:, t, 0:1], psc[:, 1:2], gnw_s[:, t:t + 1])
            tmp2 = gnp.tile([128, 1], F32, tag="ctmp", name="tmp2")
            nc.vector.tensor_mul(tmp2[:, :], psc[:, 0:1], ad[:, t, 0:1])
            nc.vector.tensor_sub(ad[:, t, 1:2], gnb_s[:, t:t + 1], tmp2[:, :])

        h_t = hp.tile([128, CT, HW], BF16, tag="h", name="h_t")
        for t in range(2):
            nc.vector.tensor_scalar(
                out=h_t[:, t, :], in0=x_t[:, t, :],
                scalar1=ad[:, t, 0:1], scalar2=ad[:, t, 1:2],
                op0=mybir.AluOpType.mult, op1=mybir.AluOpType.add,
            )
        for t in range(2, CT):
            nc.scalar.activation(
                out=h_t[:, t, :], in_=x_t[:, t, :],
                func=mybir.ActivationFunctionType.Identity,
                bias=ad[:, t, 1:2], scale=ad[:, t, 0:1],
            )
        return h_t

    def qkv(h_t):
        q_t = qp.tile([128, CT, HW], BF16, tag="q", name="q_t")
        k_t = kp.tile([128, CT, HW], BF16, tag="k", name="k_t")
        for dst, wn, bn in ((q_t, "wqT", "bq"), (k_t, "wkT", "bk")):
            for co in range(CT):
                ps = ps_mm.tile([128, HW], F32, tag="mm", name="ps")
                for hf in range(2):
                    for k in range(CT):
                        nc.tensor.matmul(
                            ps[:, 512 * hf:512 * (hf + 1)],
                            w_s[wn][:, k, 128 * co:128 * (co + 1)],
                            h_t[:, k, 512 * hf:512 * (hf + 1)],
                            start=(k == 0), stop=(k == CT - 1),
                        )
                nc.scalar.add(dst[:, co, :], ps[:, :], b_s[bn][:, co:co + 1])

        vT_t = vp.tile([128, QC, C], BF16, tag="vT", name="vT_t")
        for nq in range(QC):
            ps = ps_mm.tile([128, C], F32, tag="mm", name="psv")
            for k in range(CT):
                nc.tensor.matmul(
                    ps[:, :],
                    h_t[:, k, 128 * nq:128 * (nq + 1)],
                    w_s["wvT"][:, k, :],
                    start=(k == 0), stop=(k == CT - 1),
                )
            nc.vector.tensor_add(vT_t[:, nq, :], ps[:, :], bv_bc[:, :])
        return q_t, k_t, vT_t

    def attn_scores(q_t, k_t):
        """S^T = K_m^T Q per key-chunk; exp writes P^T directly; rowsums via
        ones-vector matmuls over the partition (key) dim, lagged one chunk."""
        pT_t = ptp.tile([128, QC, HW], BF16, tag="pT", name="pT_t")
        rs0 = ps_sm.tile([128, 512], F32, tag="sm", name="rs0")
        rs1 = ps_sm.tile([128, 512], F32, tag="sm", name="rs1")
        rs_halves = (rs0, rs1)

        def emit_rs(m):
            for hf in range(2):
                nc.tensor.matmul(
                    rs_halves[hf][:, :],
                    ones_blk[:, :128],
                    pT_t[:, m, 512 * hf:512 * (hf + 1)],
                    start=(m == 0), stop=(m == QC - 1),
                    skip_group_check=True,
                )

        for m in range(QC):
            ps = ps_mm.tile([128, HW], F32, tag="mm", name="ps_s")
            for hf in range(2):
                for k in range(CT):
                    nc.tensor.matmul(
                        ps[:, 512 * hf:512 * (hf + 1)],
                        k_t[:, k, 128 * m:128 * (m + 1)],
                        q_t[:, k, 512 * hf:512 * (hf + 1)],
                        start=(k == 0), stop=(k == CT - 1),
                    )
            nc.scalar.activation(
                out=pT_t[:, m, :], in_=ps[:, :],
                func=mybir.ActivationFunctionType.Exp, scale=SCALE)
            if m >= 1:
                emit_rs(m - 1)
        emit_rs(QC - 1)

        return pT_t, rs_halves

    def attn_apply(vT_t, pT_t, rs_halves):
        h2_t = h2p.tile([128, CT, HW], BF16, tag="h2", name="h2_t")
        for co in range(CT):
            ps = ps_mm.tile([128, HW], F32, tag="mm", name="ps_h2")
            for hf in range(2):
                for m in range(QC):
                    nc.tensor.matmul(
                        ps[:, 512 * hf:512 * (hf + 1)],
                        vT_t[:, m, 128 * co:128 * (co + 1)],
                        pT_t[:, m, 512 * hf:512 * (hf + 1)],
                        start=(m == 0), stop=(m == QC - 1),
                    )
            if co == 0:
                # rowsums arrive already replicated across partitions
                rbc_sb = rbp.tile([128, HW], F32, tag="rbc", name="rbc_sb")
                for hf in range(2):
                    nc.vector.reciprocal(
                        rbc_sb[:, 512 * hf:512 * (hf + 1)],
                        rs_halves[hf][:, :])
            nc.vector.tensor_mul(h2_t[:, co, :], ps[:, :], rbc_sb[:, :])
        return h2_t

    def out_proj(b, h2_t, x_t):
        for co in range(CT):
            ps = ps_mm.tile([128, HW], F32, tag="mm", name="ps_o")
            o_t = outp.tile([128, HW], F32, tag="o", name="o_t")
            for hf in range(2):
                sl = slice(512 * hf, 512 * (hf + 1))
                for k in range(CT):
                    nc.tensor.matmul(
                        ps[:, sl],
                        w_s["woT"][:, k, 128 * co:128 * (co + 1)],
                        h2_t[:, k, sl],
                        start=(k == 0), stop=(k == CT - 1),
                    )
                nc.vector.scalar_tensor_tensor(
                    out=o_t[:, sl], in0=ps[:, sl],
                    scalar=b_s["bo"][:, co:co + 1], in1=x_t[:, co, sl],
                    op0=mybir.AluOpType.add, op1=mybir.AluOpType.add,
                )
                nc.sync.dma_start(out=out_e[b, 128 * co:128 * (co + 1), sl],
                                  in_=o_t[:, sl])

    # ---- software-pipelined schedule over the NB batch elements ----
    # x(0) DMAs are emitted first so the stats chain starts immediately;
    # constants and weights follow on the queues behind them.
    x_t = load_x(0)
    # ---- constants / weights (loaded once) ----
    cvec_s = constp.tile([128, 5, CT], F32, tag="cvec")
    nc.gpsimd.dma_start(out=cvec_s[:, :, :], in_=ext["cvec"][:, :, :])
    b_s = {"bq": cvec_s[:, 0, :], "bk": cvec_s[:, 1, :], "bo": cvec_s[:, 2, :]}
    gnw_s = cvec_s[:, 3, :]
    gnb_s = cvec_s[:, 4, :]
    bv_bc = constp.tile([128, C], F32, tag="bv_bc")
    bv_ap = ext["bv"][:]
    nc.gpsimd.dma_start(
        out=bv_bc[:, :],
        in_=bass.AP(tensor=bv_ap.tensor, offset=bv_ap.offset,
                    ap=[[0, 128]] + list(bv_ap.ap)),
    )
    indT_s = constp.tile([128, CT, G], F32, tag="indT")
    nc.gpsimd.dma_start(out=indT_s[:, :, :], in_=ext["indT"][:, :, :])
    ind2_s = constp.tile([G, CT, 128], F32, tag="ind2")
    nc.gpsimd.dma_start(out=ind2_s[:, :, :], in_=ext["ind2"][:, :, :])
    eps_t = constp.tile([G, 1], F32, tag="eps")
    nc.vector.memset(eps_t[:, :], EPS)
    ones_blk = constp.tile([128, 512], BF16, tag="ones_blk")
    nc.vector.memset(ones_blk[:, :], 1.0)

    w_s = {}
    for wn, eng in (("wqT", nc.sync), ("wkT", nc.sync),
                    ("wvT", nc.sync), ("woT", nc.sync)):
        w_s[wn] = wtsp.tile([128, CT, C], BF16, tag=wn, name=wn)
        eng.dma_start(
            out=w_s[wn][:, :, :],
            in_=ext[wn][:, :].rearrange("(k p) c -> p k c", p=128),
        )
    h_t = gn_stats(x_t)
    nxt = None
    for b in range(NB):
        q_t, k_t, vT_t = qkv(h_t)
        if b + 1 < NB:
            x_nxt = load_x(b + 1)
            h_next = gn_stats(x_nxt)
        pT_t, rs_halves = attn_scores(q_t, k_t)
        h2_t = attn_apply(vT_t, pT_t, rs_halves)
        out_proj(b, h2_t, x_t)
        if b + 1 < NB:
            x_t = x_nxt
            h_t = h_next

    for p in reversed(list(pools.values())):
        p.release()


def build_nc():
    nc = bacc.Bacc("TRN2", target_bir_lowering=False, debug=False,
                   enable_asserts=False, num_devices=N_CORES)
    ext = {}
    ext["x"] = nc.declare_dram_parameter("x", [NB, C, HW], F32, isOutput=False)
    for wn in ("wqT", "wkT", "wvT", "woT"):
        ext[wn] = nc.declare_dram_parameter(wn, [C, C], BF16, isOutput=False)
    ext["bv"] = nc.declare_dram_parameter("bv", [C], F32, isOutput=False)
    ext["cvec"] = nc.declare_dram_parameter("cvec", [128, 5, CT], F32,
                                            isOutput=False)
    ext["indT"] = nc.declare_dram_parameter("indT", [128, CT, G], F32,
                                            isOutput=False)
    ext["ind2"] = nc.declare_dram_parameter("ind2", [G, CT, 128], F32,
                                            isOutput=False)
    ext["out"] = nc.declare_dram_parameter("out", [NB, C, HW], F32,
                                           isOutput=True)
    with tile.TileContext(nc) as tc:
        _build_body(nc, tc, ext)
    nc.compile()
    return nc


def _make_in_maps(x, gn_w, gn_b, wq, bq, wk, bk, wv, bv, wo, bo):
    xf = np.ascontiguousarray(np.asarray(x, np.float32).reshape(B, C, HW))
    indT = np.zeros((CT, 128, G), np.float32)
    ind2 = np.zeros((CT, G, 128), np.float32)
    for t in range(CT):
        for p in range(128):
            g = (128 * t + p) // GS
            # tiles 0,1 provide [mean, E[x^2]]; tiles 2,3 provide raw
            # [sum, sum_sq] via the scalar-engine accumulate path
            indT[t, p, g] = 1.0 / GS if t < 2 else 1.0 / (GS * HW)
            ind2[t, g, p] = 1.0
    import ml_dtypes
    bf = ml_dtypes.bfloat16
    common = {
        "wqT": np.ascontiguousarray(np.asarray(wq, np.float32).T.astype(bf)),
        "wkT": np.ascontiguousarray(np.asarray(wk, np.float32).T.astype(bf)),
        "wvT": np.ascontiguousarray(np.asarray(wv, np.float32).T.astype(bf)),
        "woT": np.ascontiguousarray(np.asarray(wo, np.float32).T.astype(bf)),
        "bv": np.asarray(bv, np.float32),
        "cvec": np.ascontiguousarray(np.stack([
            np.asarray(bq, np.float32), np.asarray(bk, np.float32),
            np.asarray(bo, np.float32), np.asarray(gn_w, np.float32),
            np.asarray(gn_b, np.float32)]).reshape(5, CT, 128)
            .transpose(2, 0, 1)),
        "indT": np.ascontiguousarray(indT.transpose(1, 0, 2)),
        "ind2": np.ascontiguousarray(ind2.transpose(1, 0, 2)),
    }
    return [dict(common, x=np.ascontiguousarray(xf[i * NB:(i + 1) * NB]))
            for i in range(N_CORES)]


def run(trace=False, **inputs):
    nc = build_nc()
    in_maps = _make_in_maps(**inputs)
    res = run_bass_kernel_spmd(nc, in_maps, core_ids=list(range(N_CORES)),
                               trace=trace)
    out = np.concatenate([r["out"] for r in res.results], axis=0)
    return out.reshape(B, C, H, W), res


def kernel(**inputs):
    out, _ = run(trace=False, **inputs)
    return out


if __name__ == "__main__":
    import reference

    inputs = {k: np.asarray(v) for k, v in reference.setup_inputs().items()}
    out = kernel(**inputs)
    print(out.shape, out.dtype)



# revision 4
# speedup vs baseline: 1.3671x; 1.3671x over previous
"""Trainium2 Bass kernel for nn_Attention_29326036697518.

Dense spatial self-attention block (GroupNorm -> QKV 1x1conv -> HW x HW
attention -> out-proj -> residual) over x[32, 512, 32, 32].

Sharding: pure data-parallel over the batch dim — 4 batch elements per
NeuronCore, weights replicated, no collectives.

Per-core layout (per batch element, N = H*W = 1024, C = 512):
  x, out              : [C, N] as 4 partition-tiles [128, N]   (fp32)
  h, Q, K, h2         : [C, N] as 4 partition-tiles [128, N]   (fp8e4)
  V^T                 : [N, C] as 8 partition-tiles [128, C]   (fp8e4)
  P^T = exp(S^T-SHIFT): [N, N] as 8 partition-tiles [128, N]   (fp8e4)

All heavy matmuls run in fp8e4 with perf_mode=DoubleRow (2 fp8 weights
per PE cell -> 256-deep contraction per instruction, ~1.4x bf16
throughput at FD=512). Precision is recovered by the residual path: the
attention branch is ~20x smaller than x, so fp8's ~4% relative error
lands at ~7e-3 on the final output (validated against a host emulation).

Scaling scheme (softmax is shift/scale invariant, so constants cancel):
  weights are pre-scaled by WS=16 on the host to center them in fp8's
  normal range; the 1/WS is folded into the PSUM->SBUF activation copy.
  exp() is computed as exp(S*scale - SHIFT) so P^T stays below fp8's
  240 max (max observed score ~6.8).  h2 is written to fp8 as
  (h2 * H2S) / rowsum, and the out-proj copy divides by WS*H2S.

Transpose-free attention: scores are computed directly transposed
(S^T = K_m^T Q per key-chunk), exp writes P^T in place, softmax row-sums
come from ones-vector DoubleRow matmuls over the partition dim
(replicated across partitions), and the 1/rowsum normalization is folded
into the h2 PSUM->SBUF copy (reciprocal_approx_fast). GroupNorm group
statistics are reduced across channel-partitions with tiny indicator
matmuls; stats/apply are split across the vector and scalar engines.
"""

import sys

if "/opt/trn_rl_repo" not in sys.path:
    sys.path.insert(0, "/opt/trn_rl_repo")

import numpy as np

import concourse.bass as bass
import concourse.tile as tile
from concourse import bacc, mybir
from concourse.bass_utils import run_bass_kernel_spmd

F32 = mybir.dt.float32
BF16 = mybir.dt.bfloat16
F8 = mybir.dt.float8e4
DR = mybir.MatmulPerfMode.DoubleRow

N_CORES = 8
B, C, H, W = 32, 512, 32, 32
HW = H * W                    # 1024
NB = B // N_CORES             # 4 batch elements per core
CT = C // 128                 # 4 channel partition-tiles
QC = HW // 128                # 8 spatial partition-tiles
G = 32                        # groupnorm groups
GS = C // G                   # 16 channels per group
EPS = 1e-5
SCALE = float(C) ** -0.5
WS = 16.0                     # host-side weight prescale for fp8 range
SHIFT = 5.0                   # exp(S - SHIFT): keeps P^T below fp8 max
H2S = 4.0                     # h2 prescale for fp8 range


def _build_body(nc, tc, ext):
    x_e, out_e = ext["x"], ext["out"]

    pools = {}
    def pool(name, bufs, space="SBUF"):
        pools[name] = tc.alloc_tile_pool(name=name, bufs=bufs, space=space)
        return pools[name]

    constp = pool("const", 1)
    wtsp = pool("wts", 1)
    xp = pool("xp", 2)
    hp = pool("hp", 2)
    qp = pool("qp", 1)
    kp = pool("kp", 1)
    vp = pool("vp", 1)
    ptp = pool("ptp", 1)
    h2p = pool("h2p", 1)
    outp = pool("outp", 2)
    rbp = pool("rbp", 2)
    gnp = pool("gnp", 2)
    ps_mm = pool("ps_mm", 3, space="PSUM")
    ps_sm = pool("ps_sm", 2, space="PSUM")

    def load_x(b):
        x_t = xp.tile([128, CT, HW], F32, tag="x", name="x_t")
        for t in range(CT):
            nc.sync.dma_start(out=x_t[:, t, :],
                              in_=x_e[b, 128 * t:128 * (t + 1), :])
        return x_t

    def gn_stats(x_t):
        """Groupnorm coefficients a,d; stats on DVE (tiles 0,1) + ACT (2,3),
        then group-reduce/broadcast matmuls, then h = a*x + d (fp8 out)."""
        stat2 = gnp.tile([128, 2, 2], F32, tag="stat2", name="stat2")
        stat2b = gnp.tile([128, 2, 2], F32, tag="stat2b", name="stat2b")
        for t in range(2):
            st = gnp.tile([128, 2, 6], F32, tag="bnst", name="st")
            xin = x_t[:, t, :].rearrange("p (s f) -> p s f", f=512)
            for s in range(2):
                nc.vector.bn_stats(out=st[:, s, :], in_=xin[:, s, :])
            mv = gnp.tile([128, 2], F32, tag="bnmv", name="mv")
            nc.vector.bn_aggr(out=mv[:, :], in_=st[:, :, :])
            nc.vector.tensor_copy(stat2[:, t, 0:1], mv[:, 0:1])
            nc.vector.tensor_mul(stat2[:, t, 1:2], mv[:, 0:1], mv[:, 0:1])
            nc.vector.tensor_add(stat2[:, t, 1:2], stat2[:, t, 1:2], mv[:, 1:2])
        for t in range(2, CT):
            junk = gnp.tile([128, HW], BF16, tag="junk", name="junk")
            nc.scalar.activation(
                out=junk[:, :], in_=x_t[:, t, :],
                func=mybir.ActivationFunctionType.Identity,
                accum_out=stat2b[:, t - 2, 0:1])
            nc.scalar.activation(
                out=junk[:, :], in_=x_t[:, t, :],
                func=mybir.ActivationFunctionType.Square,
                accum_out=stat2b[:, t - 2, 1:2])

        psg = ps_sm.tile([G, 2], F32, tag="sm", name="psg")
        for t in range(CT):
            s2 = stat2[:, t, :] if t < 2 else stat2b[:, t - 2, :]
            nc.tensor.matmul(
                psg[:, :], indT_s[:, t, :], s2,
                start=(t == 0), stop=(t == CT - 1),
            )
        gsb = gnp.tile([G, 2], F32, tag="gsb", name="gsb")
        nc.vector.tensor_copy(gsb[:, :], psg[:, :])
        grp = gnp.tile([G, 2], F32, tag="grp", name="grp")
        nc.vector.tensor_copy(grp[:, 0:1], gsb[:, 0:1])
        tmp = gnp.tile([G, 1], F32, tag="gtmp", name="tmp")
        nc.vector.tensor_mul(tmp[:, :], gsb[:, 0:1], gsb[:, 0:1])
        nc.vector.tensor_sub(tmp[:, :], gsb[:, 1:2], tmp[:, :])
        nc.scalar.activation(tmp[:, :], tmp[:, :],
                             mybir.ActivationFunctionType.Sqrt,
                             bias=eps_t[:, :])
        nc.vector.reciprocal(grp[:, 1:2], tmp[:, :])

        ad = gnp.tile([128, CT, 2], F32, tag="ad", name="ad")
        for t in range(CT):
            psc = ps_sm.tile([128, 2], F32, tag="sm", name="psc")
            nc.tensor.matmul(psc[:, :], ind2_s[:, t, :], grp[:, :],
                             start=True, stop=True)
            nc.vector.tensor_mul(ad[:, t, 0:1], psc[:, 1:2], gnw_s[:, t:t + 1])
            tmp2 = gnp.tile([128, 1], F32, tag="ctmp", name="tmp2")
            nc.vector.tensor_mul(tmp2[:, :], psc[:, 0:1], ad[:, t, 0:1])
            nc.vector.tensor_sub(ad[:, t, 1:2], gnb_s[:, t:t + 1], tmp2[:, :])

        h_t = hp.tile([128, CT, HW], F8, tag="h", name="h_t")
        for t in range(2):
            nc.vector.tensor_scalar(
                out=h_t[:, t, :], in0=x_t[:, t, :],
                scalar1=ad[:, t, 0:1], scalar2=ad[:, t, 1:2],
                op0=mybir.AluOpType.mult, op1=mybir.AluOpType.add,
            )
        for t in range(2, CT):
            nc.scalar.activation(
                out=h_t[:, t, :], in_=x_t[:, t, :],
                func=mybir.ActivationFunctionType.Identity,
                bias=ad[:, t, 1:2], scale=ad[:, t, 0:1],
            )
        return h_t

    def qkv(h_t):
        q_t = qp.tile([128, CT, HW], F8, tag="q", name="q_t")
        k_t = kp.tile([128, CT, HW], F8, tag="k", name="k_t")
        for dst, wn, bn in ((q_t, "wqT", "bq"), (k_t, "wkT", "bk")):
            for co in range(CT):
                ps = ps_mm.tile([128, HW], F32, tag="mm", name="ps")
                for j in range(CT // 2):
                    for hf in range(2):
                        nc.tensor.matmul(
                            ps[:, 512 * hf:512 * (hf + 1)],
                            w_s[wn][:, 2 * j:2 * j + 2, 128 * co:128 * (co + 1)],
                            h_t[:, 2 * j:2 * j + 2, 512 * hf:512 * (hf + 1)],
                            start=(j == 0), stop=(j == CT // 2 - 1),
                            perf_mode=DR, skip_group_check=True,
                        )
                nc.scalar.activation(
                    out=dst[:, co, :], in_=ps[:, :],
                    func=mybir.ActivationFunctionType.Identity,
                    bias=b_s[bn][:, co:co + 1], scale=1.0 / WS)

        vT_t = vp.tile([128, QC, C], F8, tag="vT", name="vT_t")
        for nq in range(QC):
            ps = ps_mm.tile([128, C], F32, tag="mm", name="psv")
            for j in range(CT // 2):
                nc.tensor.matmul(
                    ps[:, :],
                    h_t[:, 2 * j:2 * j + 2, 128 * nq:128 * (nq + 1)],
                    w_s["wvT"][:, 2 * j:2 * j + 2, :],
                    start=(j == 0), stop=(j == CT // 2 - 1),
                    perf_mode=DR,
                )
            nc.vector.scalar_tensor_tensor(
                out=vT_t[:, nq, :], in0=ps[:, :], scalar=1.0 / WS,
                in1=bv_bc[:, :],
                op0=mybir.AluOpType.mult, op1=mybir.AluOpType.add,
            )
        return q_t, k_t, vT_t

    def attn_scores(q_t, k_t):
        """S^T = K_m^T Q per key-chunk; exp writes P^T directly; rowsums via
        ones-vector DoubleRow matmuls over the partition dim, emitted per
        completed pair of key-chunks."""
        pT_t = ptp.tile([128, QC, HW], F8, tag="pT", name="pT_t")
        rs0 = ps_sm.tile([128, 512], F32, tag="sm", name="rs0")
        rs1 = ps_sm.tile([128, 512], F32, tag="sm", name="rs1")
        rs_halves = (rs0, rs1)

        def emit_rs(j):
            for hf in range(2):
                nc.tensor.matmul(
                    rs_halves[hf][:, :],
                    ones2[:, :, :],
                    pT_t[:, 2 * j:2 * j + 2, 512 * hf:512 * (hf + 1)],
                    start=(j == 0), stop=(j == QC // 2 - 1),
                    perf_mode=DR, skip_group_check=True,
                )

        for m in range(QC):
            ps = ps_mm.tile([128, HW], F32, tag="mm", name="ps_s")
            for j in range(CT // 2):
                for hf in range(2):
                    nc.tensor.matmul(
                        ps[:, 512 * hf:512 * (hf + 1)],
                        k_t[:, 2 * j:2 * j + 2, 128 * m:128 * (m + 1)],
                        q_t[:, 2 * j:2 * j + 2, 512 * hf:512 * (hf + 1)],
                        start=(j == 0), stop=(j == CT // 2 - 1),
                        perf_mode=DR, skip_group_check=True,
                    )
            nc.scalar.activation(
                out=pT_t[:, m, :], in_=ps[:, :],
                func=mybir.ActivationFunctionType.Exp,
                scale=SCALE, bias=nshift_t[:, :])
            if m % 2 == 1:
                emit_rs(m // 2)

        return pT_t, rs_halves

    def attn_apply(vT_t, pT_t, rs_halves):
        h2_t = h2p.tile([128, CT, HW], F8, tag="h2", name="h2_t")
        for co in range(CT):
            ps = ps_mm.tile([128, HW], F32, tag="mm", name="ps_h2")
            for j in range(QC // 2):
                for hf in range(2):
                    nc.tensor.matmul(
                        ps[:, 512 * hf:512 * (hf + 1)],
                        vT_t[:, 2 * j:2 * j + 2, 128 * co:128 * (co + 1)],
                        pT_t[:, 2 * j:2 * j + 2, 512 * hf:512 * (hf + 1)],
                        start=(j == 0), stop=(j == QC // 2 - 1),
                        perf_mode=DR, skip_group_check=True,
                    )
            if co == 0:
                # rowsums arrive already replicated across partitions
                rbc_sb = rbp.tile([128, HW], F32, tag="rbc", name="rbc_sb")
                for hf in range(2):
                    nc.vector.reciprocal_approx_fast(
                        out=rbc_sb[:, 512 * hf:512 * (hf + 1)],
                        in_=rs_halves[hf][:, :])
            nc.vector.scalar_tensor_tensor(
                out=h2_t[:, co, :], in0=ps[:, :], scalar=H2S,
                in1=rbc_sb[:, :],
                op0=mybir.AluOpType.mult, op1=mybir.AluOpType.mult,
            )
        return h2_t

    def out_proj(b, h2_t, x_t):
        for co in range(CT):
            ps = ps_mm.tile([128, HW], F32, tag="mm", name="ps_o")
            o_sb = outp.tile([128, HW], F32, tag="osb", name="o_sb")
            o_t = outp.tile([128, HW], F32, tag="o", name="o_t")
            for j in range(CT // 2):
                for hf in range(2):
                    nc.tensor.matmul(
                        ps[:, 512 * hf:512 * (hf + 1)],
                        w_s["woT"][:, 2 * j:2 * j + 2, 128 * co:128 * (co + 1)],
                        h2_t[:, 2 * j:2 * j + 2, 512 * hf:512 * (hf + 1)],
                        start=(j == 0), stop=(j == CT // 2 - 1),
                        perf_mode=DR, skip_group_check=True,
                    )
            nc.scalar.activation(
                out=o_sb[:, :], in_=ps[:, :],
                func=mybir.ActivationFunctionType.Identity,
                bias=b_s["bo"][:, co:co + 1], scale=1.0 / (WS * H2S))
            nc.vector.tensor_add(o_t[:, :], o_sb[:, :], x_t[:, co, :])
            nc.sync.dma_start(out=out_e[b, 128 * co:128 * (co + 1), :],
                              in_=o_t[:, :])

    # ---- software-pipelined schedule over the NB batch elements ----
    # x(0) DMAs are emitted first so the stats chain starts immediately;
    # constants and weights follow on the queues behind them.
    x_t = load_x(0)
    # ---- constants / weights (loaded once) ----
    cvec_s = constp.tile([128, 5, CT], F32, tag="cvec")
    nc.gpsimd.dma_start(out=cvec_s[:, :, :], in_=ext["cvec"][:, :, :])
    b_s = {"bq": cvec_s[:, 0, :], "bk": cvec_s[:, 1, :], "bo": cvec_s[:, 2, :]}
    gnw_s = cvec_s[:, 3, :]
    gnb_s = cvec_s[:, 4, :]
    bv_bc = constp.tile([128, C], F32, tag="bv_bc")
    bv_ap = ext["bv"][:]
    nc.gpsimd.dma_start(
        out=bv_bc[:, :],
        in_=bass.AP(tensor=bv_ap.tensor, offset=bv_ap.offset,
                    ap=[[0, 128]] + list(bv_ap.ap)),
    )
    indT_s = constp.tile([128, CT, G], F32, tag="indT")
    nc.gpsimd.dma_start(out=indT_s[:, :, :], in_=ext["indT"][:, :, :])
    ind2_s = constp.tile([G, CT, 128], F32, tag="ind2")
    nc.gpsimd.dma_start(out=ind2_s[:, :, :], in_=ext["ind2"][:, :, :])
    eps_t = constp.tile([G, 1], F32, tag="eps")
    nc.vector.memset(eps_t[:, :], EPS)
    nshift_t = constp.tile([128, 1], F32, tag="nshift")
    nc.vector.memset(nshift_t[:, :], -SHIFT)
    ones2 = constp.tile([128, 2, 128], F8, tag="ones2")
    nc.vector.memset(ones2[:, :, :], 1.0)

    w_s = {}
    for wn in ("wqT", "wkT", "wvT", "woT"):
        w_s[wn] = wtsp.tile([128, CT, C], F8, tag=wn, name=wn)
        nc.sync.dma_start(
            out=w_s[wn][:, :, :],
            in_=ext[wn][:, :].rearrange("(k p) c -> p k c", p=128),
        )
    h_t = gn_stats(x_t)
    for b in range(NB):
        q_t, k_t, vT_t = qkv(h_t)
        if b + 1 < NB:
            x_nxt = load_x(b + 1)
            h_next = gn_stats(x_nxt)
        pT_t, rs_halves = attn_scores(q_t, k_t)
        h2_t = attn_apply(vT_t, pT_t, rs_halves)
        out_proj(b, h2_t, x_t)
        if b + 1 < NB:
            x_t = x_nxt
            h_t = h_next

    for p in reversed(list(pools.values())):
        p.release()


def build_nc():
    nc = bacc.Bacc("TRN2", target_bir_lowering=False, debug=False,
                   enable_asserts=False, num_devices=N_CORES)
    ext = {}
    ext["x"] = nc.declare_dram_parameter("x", [NB, C, HW], F32, isOutput=False)
    for wn in ("wqT", "wkT", "wvT", "woT"):
        ext[wn] = nc.declare_dram_parameter(wn, [C, C], F8, isOutput=False)
    ext["bv"] = nc.declare_dram_parameter("bv", [C], F32, isOutput=False)
    ext["cvec"] = nc.declare_dram_parameter("cvec", [128, 5, CT], F32,
                                            isOutput=False)
    ext["indT"] = nc.declare_dram_parameter("indT", [128, CT, G], F32,
                                            isOutput=False)
    ext["ind2"] = nc.declare_dram_parameter("ind2", [G, CT, 128], F32,
                                            isOutput=False)
    ext["out"] = nc.declare_dram_parameter("out", [NB, C, HW], F32,
                                           isOutput=True)
    with tile.TileContext(nc) as tc:
        _build_body(nc, tc, ext)
    nc.compile()
    return nc


def _make_in_maps(x, gn_w, gn_b, wq, bq, wk, bk, wv, bv, wo, bo):
    xf = np.ascontiguousarray(np.asarray(x, np.float32).reshape(B, C, HW))
    indT = np.zeros((CT, 128, G), np.float32)
    ind2 = np.zeros((CT, G, 128), np.float32)
    for t in range(CT):
        for p in range(128):
            g = (128 * t + p) // GS
            # tiles 0,1 provide [mean, E[x^2]]; tiles 2,3 provide raw
            # [sum, sum_sq] via the scalar-engine accumulate path
            indT[t, p, g] = 1.0 / GS if t < 2 else 1.0 / (GS * HW)
            ind2[t, g, p] = 1.0
    import ml_dtypes
    f8 = ml_dtypes.float8_e4m3fn

    def wq8(w):
        wT = np.asarray(w, np.float32).T * WS
        # TRN fp8e4 diverges from OCP e4m3fn above 240 (Inf/NaN region)
        return np.ascontiguousarray(np.clip(wT, -240.0, 240.0).astype(f8))

    common = {
        "wqT": wq8(wq),
        "wkT": wq8(wk),
        "wvT": wq8(wv),
        "woT": wq8(wo),
        "bv": np.asarray(bv, np.float32),
        "cvec": np.ascontiguousarray(np.stack([
            np.asarray(bq, np.float32), np.asarray(bk, np.float32),
            np.asarray(bo, np.float32), np.asarray(gn_w, np.float32),
            np.asarray(gn_b, np.float32)]).reshape(5, CT, 128)
            .transpose(2, 0, 1)),
        "indT": np.ascontiguousarray(indT.transpose(1, 0, 2)),
        "ind2": np.ascontiguousarray(ind2.transpose(1, 0, 2)),
    }
    return [dict(common, x=np.ascontiguousarray(xf[i * NB:(i + 1) * NB]))
            for i in range(N_CORES)]


def run(trace=False, **inputs):
    nc = build_nc()
    in_maps = _make_in_maps(**inputs)
    res = run_bass_kernel_spmd(nc, in_maps, core_ids=list(range(N_CORES)),
                               trace=trace)
    out = np.concatenate([r["out"] for r in res.results], axis=0)
    return out.reshape(B, C, H, W), res


def kernel(**inputs):
    out, _ = run(trace=False, **inputs)
    return out


if __name__ == "__main__":
    import reference

    inputs = {k: np.asarray(v) for k, v in reference.setup_inputs().items()}
    out = kernel(**inputs)
    print(out.shape, out.dtype)


# revision 5
# speedup vs baseline: 1.3677x; 1.0005x over previous
"""Trainium2 Bass kernel for nn_Attention_29326036697518.

Dense spatial self-attention block (GroupNorm -> QKV 1x1conv -> HW x HW
attention -> out-proj -> residual) over x[32, 512, 32, 32].

Sharding: pure data-parallel over the batch dim — 4 batch elements per
NeuronCore, weights replicated, no collectives.

Per-core layout (per batch element, N = H*W = 1024, C = 512):
  x, out              : [C, N] as 4 partition-tiles [128, N]   (fp32)
  h, Q, K, h2         : [C, N] as 4 partition-tiles [128, N]   (fp8e4)
  V^T                 : [N, C] as 8 partition-tiles [128, C]   (fp8e4)
  P^T = exp(S^T-SHIFT): [N, N] as 8 partition-tiles [128, N]   (fp8e4)

All heavy matmuls run in fp8e4 with perf_mode=DoubleRow (2 fp8 weights
per PE cell -> 256-deep contraction per instruction, ~2x bf16 MACs at
the same 1 column/cycle stream rate). Precision is recovered by the
residual path: the attention branch is ~20x smaller than x, so fp8's
~4% relative error lands at ~6e-3 on the final output (validated
against a host emulation).

Scaling scheme (softmax is shift/scale invariant, so constants cancel):
  weights are pre-scaled by WS=16 on the host to center them in fp8's
  normal range; the 1/WS is folded into the PSUM->SBUF copies.
  exp() is computed as exp(S*scale - SHIFT) so P^T stays below fp8's
  240 max (max observed score ~6.8).  h2 is written to fp8 as
  (h2 * H2S) / rowsum, and the out-proj copy divides by WS*H2S.

Engine budget per batch element (~30us tensor, ~26us scalar, ~21us
vector): PSUM->SBUF copies are split ACT (q,k,out,exp) / DVE (v,h2);
residual adds run on GpSimd (SBUF-only); GroupNorm rsqrt is computed as
exp(-0.5*ln(var+eps)) so the whole kernel uses one ACT table set (Exp/
Ln/Identity/Square) and never swaps tables mid-stream. GroupNorm for
batch b+1 is emitted in three phases interleaved with batch b's
attention so its tiny PE reductions never stall the in-order PE queue,
and dummy fp8 matmuls warm the PE during the batch-0 GroupNorm.
"""

import sys

if "/opt/trn_rl_repo" not in sys.path:
    sys.path.insert(0, "/opt/trn_rl_repo")

import numpy as np

import concourse.bass as bass
import concourse.tile as tile
from concourse import bacc, mybir
from concourse.bass_utils import run_bass_kernel_spmd

F32 = mybir.dt.float32
BF16 = mybir.dt.bfloat16
F8 = mybir.dt.float8e4
DR = mybir.MatmulPerfMode.DoubleRow
AF = mybir.ActivationFunctionType

N_CORES = 8
B, C, H, W = 32, 512, 32, 32
HW = H * W                    # 1024
NB = B // N_CORES             # 4 batch elements per core
CT = C // 128                 # 4 channel partition-tiles
QC = HW // 128                # 8 spatial partition-tiles
G = 32                        # groupnorm groups
GS = C // G                   # 16 channels per group
EPS = 1e-5
SCALE = float(C) ** -0.5
WS = 16.0                     # host-side weight prescale for fp8 range
SHIFT = 5.0                   # exp(S - SHIFT): keeps P^T below fp8 max
H2S = 4.0                     # h2 prescale for fp8 range


def _build_body(nc, tc, ext):
    x_e, out_e = ext["x"], ext["out"]

    pools = {}
    def pool(name, bufs, space="SBUF"):
        pools[name] = tc.alloc_tile_pool(name=name, bufs=bufs, space=space)
        return pools[name]

    constp = pool("const", 1)
    wtsp = pool("wts", 1)
    xp = pool("xp", 2)
    hp = pool("hp", 2)
    qp = pool("qp", 1)
    kp = pool("kp", 1)
    vp = pool("vp", 1)
    ptp = pool("ptp", 1)
    h2p = pool("h2p", 1)
    outp = pool("outp", 2)
    rbp = pool("rbp", 2)
    gnp = pool("gnp", 2)
    ps_mm = pool("ps_mm", 3, space="PSUM")
    ps_sm = pool("ps_sm", 2, space="PSUM")

    def load_x(b):
        x_t = xp.tile([128, CT, HW], F32, tag="x", name="x_t")
        for t in range(CT):
            nc.sync.dma_start(out=x_t[:, t, :],
                              in_=x_e[b, 128 * t:128 * (t + 1), :])
        return x_t

    def gn_pt1(x_t, use_act):
        """Per-channel [mean, E[x^2]] into stat2[128, CT, 2].

        DVE bn_stats path for all tiles; for the prologue batch the last
        two tiles go through the ACT accumulate path instead (halves the
        serial latency while nothing else runs)."""
        stat2 = gnp.tile([128, CT, 2], F32, tag="stat2", name="stat2")
        n_dve = 2 if use_act else CT
        for t in range(n_dve):
            st = gnp.tile([128, 2, 6], F32, tag="bnst", name="st")
            xin = x_t[:, t, :].rearrange("p (s f) -> p s f", f=512)
            for s in range(2):
                nc.vector.bn_stats(out=st[:, s, :], in_=xin[:, s, :])
            mv = gnp.tile([128, 2], F32, tag="bnmv", name="mv")
            nc.vector.bn_aggr(out=mv[:, :], in_=st[:, :, :])
            nc.vector.tensor_copy(stat2[:, t, 0:1], mv[:, 0:1])
            nc.vector.tensor_mul(stat2[:, t, 1:2], mv[:, 0:1], mv[:, 0:1])
            nc.vector.tensor_add(stat2[:, t, 1:2], stat2[:, t, 1:2], mv[:, 1:2])
        for t in range(n_dve, CT):
            junk = gnp.tile([128, HW], BF16, tag="junk", name="junk")
            acc = gnp.tile([128, 2], F32, tag="acc", name="acc")
            nc.scalar.activation(
                out=junk[:, :], in_=x_t[:, t, :], func=AF.Identity,
                accum_out=acc[:, 0:1])
            nc.scalar.activation(
                out=junk[:, :], in_=x_t[:, t, :], func=AF.Square,
                accum_out=acc[:, 1:2])
            nc.vector.tensor_scalar_mul(stat2[:, t, :], acc[:, :], 1.0 / HW)
        return stat2

    def gn_pt2a(stat2):
        """Group-reduce across channel partitions, then per-channel
        a,d coefficients.  rsqrt = exp(-0.5*ln(var+eps)) keeps the ACT
        engine on the natural_log_exp table set (no table swap)."""
        psg = ps_sm.tile([G, 2], F32, tag="sm", name="psg")
        for t in range(CT):
            nc.tensor.matmul(
                psg[:, :], indT_s[:, t, :], stat2[:, t, :],
                start=(t == 0), stop=(t == CT - 1),
            )
        gsb = gnp.tile([G, 2], F32, tag="gsb", name="gsb")
        nc.vector.tensor_copy(gsb[:, :], psg[:, :])
        grp = gnp.tile([G, 2], F32, tag="grp", name="grp")
        nc.vector.tensor_copy(grp[:, 0:1], gsb[:, 0:1])
        tmp = gnp.tile([G, 1], F32, tag="gtmp", name="tmp")
        nc.vector.tensor_mul(tmp[:, :], gsb[:, 0:1], gsb[:, 0:1])
        nc.vector.tensor_sub(tmp[:, :], gsb[:, 1:2], tmp[:, :])
        nc.scalar.activation(tmp[:, :], tmp[:, :], AF.Ln, bias=eps_t[:, :])
        nc.scalar.activation(grp[:, 1:2], tmp[:, :], AF.Exp, scale=-0.5)

        ad = gnp.tile([128, CT, 2], F32, tag="ad", name="ad")
        for t in range(CT):
            psc = ps_sm.tile([128, 2], F32, tag="sm", name="psc")
            nc.tensor.matmul(psc[:, :], ind2_s[:, t, :], grp[:, :],
                             start=True, stop=True)
            nc.vector.tensor_mul(ad[:, t, 0:1], psc[:, 1:2], gnw_s[:, t:t + 1])
            tmp2 = gnp.tile([128, 1], F32, tag="ctmp", name="tmp2")
            nc.vector.tensor_mul(tmp2[:, :], psc[:, 0:1], ad[:, t, 0:1])
            nc.vector.tensor_sub(ad[:, t, 1:2], gnb_s[:, t:t + 1], tmp2[:, :])
        return ad

    def gn_pt2b(x_t, ad):
        """h = a*x + d, fp8 out; tiles 0,1 on DVE, 2,3 on ACT."""
        h_t = hp.tile([128, CT, HW], F8, tag="h", name="h_t")
        for t in range(2):
            nc.vector.tensor_scalar(
                out=h_t[:, t, :], in0=x_t[:, t, :],
                scalar1=ad[:, t, 0:1], scalar2=ad[:, t, 1:2],
                op0=mybir.AluOpType.mult, op1=mybir.AluOpType.add,
            )
        for t in range(2, CT):
            nc.scalar.activation(
                out=h_t[:, t, :], in_=x_t[:, t, :], func=AF.Identity,
                bias=ad[:, t, 1:2], scale=ad[:, t, 0:1],
            )
        return h_t

    def qkv(h_t):
        q_t = qp.tile([128, CT, HW], F8, tag="q", name="q_t")
        k_t = kp.tile([128, CT, HW], F8, tag="k", name="k_t")
        for dst, wn, bn in ((q_t, "wqT", "bq"), (k_t, "wkT", "bk")):
            for co in range(CT):
                ps = ps_mm.tile([128, HW], F32, tag="mm", name="ps")
                for j in range(CT // 2):
                    for hf in range(2):
                        nc.tensor.matmul(
                            ps[:, 512 * hf:512 * (hf + 1)],
                            w_s[wn][:, 2 * j:2 * j + 2, 128 * co:128 * (co + 1)],
                            h_t[:, 2 * j:2 * j + 2, 512 * hf:512 * (hf + 1)],
                            start=(j == 0), stop=(j == CT // 2 - 1),
                            perf_mode=DR, skip_group_check=True,
                        )
                nc.scalar.activation(
                    out=dst[:, co, :], in_=ps[:, :], func=AF.Identity,
                    bias=b_s[bn][:, co:co + 1], scale=1.0 / WS)

        vT_t = vp.tile([128, QC, C], F8, tag="vT", name="vT_t")
        for nq in range(QC):
            ps = ps_mm.tile([128, C], F32, tag="mm", name="psv")
            for j in range(CT // 2):
                nc.tensor.matmul(
                    ps[:, :],
                    h_t[:, 2 * j:2 * j + 2, 128 * nq:128 * (nq + 1)],
                    w_s["wvT"][:, 2 * j:2 * j + 2, :],
                    start=(j == 0), stop=(j == CT // 2 - 1),
                    perf_mode=DR,
                )
            nc.vector.scalar_tensor_tensor(
                out=vT_t[:, nq, :], in0=ps[:, :], scalar=1.0 / WS,
                in1=bv_bc[:, :],
                op0=mybir.AluOpType.mult, op1=mybir.AluOpType.add,
            )
        return q_t, k_t, vT_t

    def attn_scores(q_t, k_t):
        """S^T = K_m^T Q per key-chunk; exp writes P^T directly; rowsums via
        ones-vector DoubleRow matmuls over the partition dim, emitted per
        completed pair of key-chunks."""
        pT_t = ptp.tile([128, QC, HW], F8, tag="pT", name="pT_t")
        rs0 = ps_sm.tile([128, 512], F32, tag="sm", name="rs0")
        rs1 = ps_sm.tile([128, 512], F32, tag="sm", name="rs1")
        rs_halves = (rs0, rs1)

        def emit_rs(j):
            for hf in range(2):
                nc.tensor.matmul(
                    rs_halves[hf][:, :],
                    ones2[:, :, :],
                    pT_t[:, 2 * j:2 * j + 2, 512 * hf:512 * (hf + 1)],
                    start=(j == 0), stop=(j == QC // 2 - 1),
                    perf_mode=DR, skip_group_check=True,
                )

        for m in range(QC):
            ps = ps_mm.tile([128, HW], F32, tag="mm", name="ps_s")
            for j in range(CT // 2):
                for hf in range(2):
                    nc.tensor.matmul(
                        ps[:, 512 * hf:512 * (hf + 1)],
                        k_t[:, 2 * j:2 * j + 2, 128 * m:128 * (m + 1)],
                        q_t[:, 2 * j:2 * j + 2, 512 * hf:512 * (hf + 1)],
                        start=(j == 0), stop=(j == CT // 2 - 1),
                        perf_mode=DR, skip_group_check=True,
                    )
            nc.scalar.activation(
                out=pT_t[:, m, :], in_=ps[:, :], func=AF.Exp,
                scale=SCALE, bias=nshift_t[:, :])
            if m % 2 == 1:
                emit_rs(m // 2)

        return pT_t, rs_halves

    def attn_apply(vT_t, pT_t, rs_halves):
        h2_t = h2p.tile([128, CT, HW], F8, tag="h2", name="h2_t")
        for co in range(CT):
            ps = ps_mm.tile([128, HW], F32, tag="mm", name="ps_h2")
            for j in range(QC // 2):
                for hf in range(2):
                    nc.tensor.matmul(
                        ps[:, 512 * hf:512 * (hf + 1)],
                        vT_t[:, 2 * j:2 * j + 2, 128 * co:128 * (co + 1)],
                        pT_t[:, 2 * j:2 * j + 2, 512 * hf:512 * (hf + 1)],
                        start=(j == 0), stop=(j == QC // 2 - 1),
                        perf_mode=DR, skip_group_check=True,
                    )
            if co == 0:
                # rowsums arrive already replicated across partitions
                rbc_sb = rbp.tile([128, HW], F32, tag="rbc", name="rbc_sb")
                for hf in range(2):
                    nc.vector.reciprocal_approx_fast(
                        out=rbc_sb[:, 512 * hf:512 * (hf + 1)],
                        in_=rs_halves[hf][:, :])
            nc.vector.scalar_tensor_tensor(
                out=h2_t[:, co, :], in0=ps[:, :], scalar=H2S,
                in1=rbc_sb[:, :],
                op0=mybir.AluOpType.mult, op1=mybir.AluOpType.mult,
            )
        return h2_t

    def out_proj(b, h2_t, x_t):
        for co in range(CT):
            ps = ps_mm.tile([128, HW], F32, tag="mm", name="ps_o")
            o_sb = outp.tile([128, HW], F32, tag="osb", name="o_sb")
            o_t = outp.tile([128, HW], F32, tag="o", name="o_t")
            for j in range(CT // 2):
                for hf in range(2):
                    nc.tensor.matmul(
                        ps[:, 512 * hf:512 * (hf + 1)],
                        w_s["woT"][:, 2 * j:2 * j + 2, 128 * co:128 * (co + 1)],
                        h2_t[:, 2 * j:2 * j + 2, 512 * hf:512 * (hf + 1)],
                        start=(j == 0), stop=(j == CT // 2 - 1),
                        perf_mode=DR, skip_group_check=True,
                    )
            nc.scalar.activation(
                out=o_sb[:, :], in_=ps[:, :], func=AF.Identity,
                bias=b_s["bo"][:, co:co + 1], scale=1.0 / (WS * H2S))
            nc.gpsimd.tensor_add(o_t[:, :], o_sb[:, :], x_t[:, co, :])
            nc.sync.dma_start(out=out_e[b, 128 * co:128 * (co + 1), :],
                              in_=o_t[:, :])

    # ---- prologue: x(0) DMAs first so the stats chain starts at once ----
    x_t = load_x(0)
    # ---- constants / weights (loaded once) ----
    cvec_s = constp.tile([128, 5, CT], F32, tag="cvec")
    nc.gpsimd.dma_start(out=cvec_s[:, :, :], in_=ext["cvec"][:, :, :])
    b_s = {"bq": cvec_s[:, 0, :], "bk": cvec_s[:, 1, :], "bo": cvec_s[:, 2, :]}
    gnw_s = cvec_s[:, 3, :]
    gnb_s = cvec_s[:, 4, :]
    bv_bc = constp.tile([128, C], F32, tag="bv_bc")
    bv_ap = ext["bv"][:]
    nc.gpsimd.dma_start(
        out=bv_bc[:, :],
        in_=bass.AP(tensor=bv_ap.tensor, offset=bv_ap.offset,
                    ap=[[0, 128]] + list(bv_ap.ap)),
    )
    indT_s = constp.tile([128, CT, G], F32, tag="indT")
    nc.gpsimd.dma_start(out=indT_s[:, :, :], in_=ext["indT"][:, :, :])
    ind2_s = constp.tile([G, CT, 128], F32, tag="ind2")
    nc.gpsimd.dma_start(out=ind2_s[:, :, :], in_=ext["ind2"][:, :, :])
    eps_t = constp.tile([G, 1], F32, tag="eps")
    nc.vector.memset(eps_t[:, :], EPS)
    nshift_t = constp.tile([128, 1], F32, tag="nshift")
    nc.vector.memset(nshift_t[:, :], -SHIFT)
    ones2 = constp.tile([128, 2, 128], F8, tag="ones2")
    nc.vector.memset(ones2[:, :, :], 1.0)
    warm = constp.tile([128, 2, 512], F8, tag="warm")
    nc.vector.memset(warm[:, :, :], 0.0)

    w_s = {}
    for wn in ("wqT", "wkT", "wvT", "woT"):
        w_s[wn] = wtsp.tile([128, CT, C], F8, tag=wn, name=wn)
        nc.sync.dma_start(
            out=w_s[wn][:, :, :],
            in_=ext[wn][:, :].rearrange("(k p) c -> p k c", p=128),
        )

    # dummy matmuls keep the PE busy/warm through the batch-0 GroupNorm
    for wi in range(32):
        wps = ps_mm.tile([128, 512], F32, tag="mm", name="warm_ps")
        nc.tensor.matmul(wps[:, :], ones2[:, :, :], warm[:, :, :],
                         start=True, stop=True, perf_mode=DR,
                         skip_group_check=True)

    stat2 = gn_pt1(x_t, use_act=True)
    ad = gn_pt2a(stat2)
    h_t = gn_pt2b(x_t, ad)
    for b in range(NB):
        q_t, k_t, vT_t = qkv(h_t)
        if b + 1 < NB:
            x_nxt = load_x(b + 1)
            stat2 = gn_pt1(x_nxt, use_act=False)
        pT_t, rs_halves = attn_scores(q_t, k_t)
        if b + 1 < NB:
            ad = gn_pt2a(stat2)
        h2_t = attn_apply(vT_t, pT_t, rs_halves)
        if b + 1 < NB:
            h_next = gn_pt2b(x_nxt, ad)
        out_proj(b, h2_t, x_t)
        if b + 1 < NB:
            x_t = x_nxt
            h_t = h_next

    for p in reversed(list(pools.values())):
        p.release()


def build_nc():
    nc = bacc.Bacc("TRN2", target_bir_lowering=False, debug=False,
                   enable_asserts=False, num_devices=N_CORES)
    ext = {}
    ext["x"] = nc.declare_dram_parameter("x", [NB, C, HW], F32, isOutput=False)
    for wn in ("wqT", "wkT", "wvT", "woT"):
        ext[wn] = nc.declare_dram_parameter(wn, [C, C], F8, isOutput=False)
    ext["bv"] = nc.declare_dram_parameter("bv", [C], F32, isOutput=False)
    ext["cvec"] = nc.declare_dram_parameter("cvec", [128, 5, CT], F32,
                                            isOutput=False)
    ext["indT"] = nc.declare_dram_parameter("indT", [128, CT, G], F32,
                                            isOutput=False)
    ext["ind2"] = nc.declare_dram_parameter("ind2", [G, CT, 128], F32,
                                            isOutput=False)
    ext["out"] = nc.declare_dram_parameter("out", [NB, C, HW], F32,
                                           isOutput=True)
    with tile.TileContext(nc) as tc:
        _build_body(nc, tc, ext)
    nc.compile()
    return nc


def _make_in_maps(x, gn_w, gn_b, wq, bq, wk, bk, wv, bv, wo, bo):
    xf = np.ascontiguousarray(np.asarray(x, np.float32).reshape(B, C, HW))
    indT = np.zeros((CT, 128, G), np.float32)
    ind2 = np.zeros((CT, G, 128), np.float32)
    for t in range(CT):
        for p in range(128):
            g = (128 * t + p) // GS
            indT[t, p, g] = 1.0 / GS   # every tile provides [mean, E[x^2]]
            ind2[t, g, p] = 1.0
    import ml_dtypes
    f8 = ml_dtypes.float8_e4m3fn

    def wq8(w):
        wT = np.asarray(w, np.float32).T * WS
        # TRN fp8e4 diverges from OCP e4m3fn above 240 (Inf/NaN region)
        return np.ascontiguousarray(np.clip(wT, -240.0, 240.0).astype(f8))

    common = {
        "wqT": wq8(wq),
        "wkT": wq8(wk),
        "wvT": wq8(wv),
        "woT": wq8(wo),
        "bv": np.asarray(bv, np.float32),
        "cvec": np.ascontiguousarray(np.stack([
            np.asarray(bq, np.float32), np.asarray(bk, np.float32),
            np.asarray(bo, np.float32), np.asarray(gn_w, np.float32),
            np.asarray(gn_b, np.float32)]).reshape(5, CT, 128)
            .transpose(2, 0, 1)),
        "indT": np.ascontiguousarray(indT.transpose(1, 0, 2)),
        "ind2": np.ascontiguousarray(ind2.transpose(1, 0, 2)),
    }
    return [dict(common, x=np.ascontiguousarray(xf[i * NB:(i + 1) * NB]))
            for i in range(N_CORES)]


def run(trace=False, **inputs):
    nc = build_nc()
    in_maps = _make_in_maps(**inputs)
    res = run_bass_kernel_spmd(nc, in_maps, core_ids=list(range(N_CORES)),
                               trace=trace)
    out = np.concatenate([r["out"] for r in res.results], axis=0)
    return out.reshape(B, C, H, W), res


def kernel(**inputs):
    out, _ = run(trace=False, **inputs)
    return out


if __name__ == "__main__":
    import reference

    inputs = {k: np.asarray(v) for k, v in reference.setup_inputs().items()}
    out = kernel(**inputs)
    print(out.shape, out.dtype)
